# revision 3
# baseline (speedup 1.0000x reference)
"""Trainium2 Bass kernel for the per-cell-MLP "MAR one-sided missingness" model.

Model (per cell (n,t) of a 1024x128 grid):
    xc     = X[n, col_idx[n,t]]
    h      = relu(W_in[n,t,:,0]*xc + W_in[n,t,:,1]*X[n,t] + b_in[n,t,:])   # [H]
    out    = sigmoid(dot(W_out[n,t,:], h) + b_out[n,t])

Sharding: rows N split across 8 cores (128 rows each), fully data parallel.

Per-core layout: partition dim = t (128 cells of one row), free dim = h.
All four per-cell weight tensors stream as ONE interleaved f16 wall
[t, n, (w0,w1,b,wo), h]; rows are processed in chunks (16 rows = one 2MB
DMA), tapering to tiny chunks at the end so the post-DMA drain is short.
The neighbor gather X[n, col_idx[n,t]] runs on the PE as a one-hot matmul
(f8 one-hot stationary x f16 X moving -> f32 xc in PSUM).

Per chunk, engines split the per-cell MLP (intermediates kept f32; only
the streamed weights are f16):
  PE   : xc[:, g] = oh_g^T @ Xmov[:, n]          (per row)
  ACT  : m0_g = w0_g * xc_g                      (Copy activation, scale/partition)
  Pool : v_g  = (w1_g * x_g) + b_g               (scalar_tensor_tensor)
  DVE  : u    = m0 + v                           (batched TT)
  DVE  : r_g  = (u_g max 0) * wo_g, accum red[:, n] = sum_h r_g   (STT+accum)
  DVE/ACT epilogue per chunk: out slice = sigmoid(red + b_out^T), DMA out.
Host transposes the [T, NR] result back to [NR, T].

HBM-bandwidth bound: streams ~18.2 MB per core (f16 weights + f8 one-hot).
"""

import ml_dtypes
import numpy as np

N, T, H = 1024, 128, 128
M = 8            # cores
NR = N // M      # rows per core
CH = 16          # max chunk rows (one 2MB wall DMA)
CHUNKS = [12] + [16] * 7 + [2, 2]
assert sum(CHUNKS) == NR

_cache = {}


def _build():
    if "nc" in _cache:
        return _cache["nc"]
    import concourse.bacc as bacc
    import concourse.mybir as mybir
    import concourse.tile as tile

    f32 = mybir.dt.float32
    f16 = mybir.dt.float16
    f8 = mybir.dt.float8e4
    Alu = mybir.AluOpType
    Act = mybir.ActivationFunctionType

    nc = bacc.Bacc()
    wall = nc.declare_dram_parameter("wall", [T, NR, 4, H], f16, isOutput=False)
    ohall = nc.declare_dram_parameter("ohall", [128, NR * T], f8, isOutput=False)
    xmov = nc.declare_dram_parameter("xmov", [128, NR], f16, isOutput=False)
    xb = nc.declare_dram_parameter("xb", [T, 2 * NR], f32, isOutput=False)
    out = nc.declare_dram_parameter("out", [T, NR], f32, isOutput=True)

    with tile.TileContext(nc) as tc:
        with (
            tc.tile_pool(name="const", bufs=1) as constp,
            tc.tile_pool(name="wallp", bufs=5) as wallp,
            tc.tile_pool(name="ohp", bufs=6) as ohp,
            tc.tile_pool(name="mid", bufs=4) as midp,
            tc.tile_pool(name="rp", bufs=1) as rp,
            tc.tile_pool(name="acc", bufs=1) as accp,
            tc.tile_pool(name="psxc", bufs=4, space="PSUM") as psxcp,
        ):
            xb_sb = constp.tile([T, 2 * NR], f32)
            xmov_sb = constp.tile([128, NR], f16)
            xt_sb = xb_sb[:, :NR]
            bo_sb = xb_sb[:, NR:]

            red = accp.tile([T, NR], f32)
            ot = accp.tile([T, NR], f32)
            r = rp.tile([128, CH * H], f16)

            # pin the {sigmoid, copy, relu} activation-function set once at
            # kernel start so no table reload lands on the critical tail
            warm = constp.tile([128, 1], f32)
            nc.vector.memset(warm[:], 0.0)
            nc.scalar.activation(ot[:, 0:1], warm[:], Act.Sigmoid)

            n0 = 0
            for ci, G in enumerate(CHUNKS):
                oh = ohp.tile([128, CH * T], f8, tag="oh")
                nc.sync.dma_start(oh[:, : G * T], ohall[:, n0 * T : (n0 + G) * T])
                wt = wallp.tile([128, CH * 4 * H], f16, tag="w")
                nc.sync.dma_start(wt[:, : G * 4 * H], wall[:, n0 : n0 + G])
                if ci == 0:
                    nc.sync.dma_start(xb_sb[:], xb[:])
                    nc.sync.dma_start(xmov_sb[:], xmov[:])
                wv = wt[:].rearrange("p (g k h) -> p g k h", g=CH, k=4)

                # neighbor gather on PE: xc[:, g] = oh_g^T @ Xmov[:, n]
                xc_ps = psxcp.tile([128, CH], f32, tag="xc")
                for g in range(G):
                    n = n0 + g
                    nc.tensor.matmul(
                        xc_ps[:, g : g + 1],
                        oh[:, g * T : (g + 1) * T],
                        xmov_sb[:, n : n + 1],
                        start=True,
                        stop=True,
                    )
                xc_sb = midp.tile([128, CH], f32, tag="xcsb")
                nc.scalar.copy(xc_sb[:, :G], xc_ps[:, :G])

                # ACT: m0_g = w0_g * xc_g
                m0 = midp.tile([128, CH * H], f32, tag="m0")
                for g in range(G):
                    nc.scalar.activation(
                        m0[:, g * H : (g + 1) * H],
                        wv[:, g, 0, :],
                        Act.Copy,
                        scale=xc_sb[:, g : g + 1],
                    )

                # DVE: v_g = (w1_g * x_g) + b_g  (STT; Pool rejects STT on HW)
                v = midp.tile([128, CH * H], f32, tag="v")
                for g in range(G):
                    n = n0 + g
                    nc.vector.scalar_tensor_tensor(
                        v[:, g * H : (g + 1) * H],
                        wv[:, g, 1, :],
                        xb_sb[:, n : n + 1],
                        wv[:, g, 2, :],
                        Alu.mult,
                        Alu.add,
                    )

                # DVE, quarter-chunk granularity so it trails ACT/Pool
                # closely: u = m0 + v, then r_g = relu(u_g) * wo_g with
                # red[:, n] = sum_h r_g
                # Pool: u = m0 + v per quarter; DVE: r_g with accum
                u = midp.tile([128, CH * H], f32, tag="u")
                for q0 in range(0, G, 4):
                    q1 = min(q0 + 4, G)
                    nc.gpsimd.tensor_tensor(
                        u[:, q0 * H : q1 * H],
                        m0[:, q0 * H : q1 * H],
                        v[:, q0 * H : q1 * H],
                        Alu.add,
                    )
                for q0 in range(0, G, 4):
                    q1 = min(q0 + 4, G)
                    for g in range(q0, q1):
                        n = n0 + g
                        nc.vector.scalar_tensor_tensor(
                            r[:, g * H : (g + 1) * H],
                            u[:, g * H : (g + 1) * H],
                            0.0,
                            wv[:, g, 3, :],
                            Alu.max,
                            Alu.mult,
                            accum_out=red[:, n : n + 1],
                        )

                n0 += G

            HALF = 64
            lg = accp.tile([T, NR], f32)
            nc.vector.tensor_tensor(
                lg[:, :HALF], red[:, :HALF], xb_sb[:, NR : NR + HALF], Alu.add
            )
            nc.scalar.activation(ot[:, :HALF], lg[:, :HALF], Act.Sigmoid)
            nc.sync.dma_start(out[:, :HALF], ot[:, :HALF])
            nc.vector.tensor_tensor(
                lg[:, HALF:], red[:, HALF:], xb_sb[:, NR + HALF :], Alu.add
            )
            nc.scalar.activation(ot[:, HALF:], lg[:, HALF:], Act.Sigmoid)
            nc.sync.dma_start(out[:, HALF:], ot[:, HALF:])

    nc.compile()
    _cache["nc"] = nc
    return nc


def make_in_maps(X, W_in, b_in, W_out, b_out, col_idx):
    X = np.asarray(X, dtype=np.float32)
    W_in = np.asarray(W_in, dtype=np.float32)
    b_in = np.asarray(b_in, dtype=np.float32)
    W_out = np.asarray(W_out, dtype=np.float32)
    b_out = np.asarray(b_out, dtype=np.float32)
    col_idx = np.asarray(col_idx)

    jj = np.arange(128)
    in_maps = []
    for c in range(M):
        sl = slice(c * NR, (c + 1) * NR)
        Wc = W_in[sl]  # [NR, T, H, 2]
        w0 = Wc[:, :, :, 0].transpose(1, 0, 2)
        w1 = Wc[:, :, :, 1].transpose(1, 0, 2)
        bb = b_in[sl].transpose(1, 0, 2)
        wo = W_out[sl].transpose(1, 0, 2)
        wallv = np.stack([w0, w1, bb, wo], axis=2).astype(np.float16)  # [T,NR,4,H]

        ohall = (col_idx[sl].reshape(1, -1) == jj[:, None]).astype(
            ml_dtypes.float8_e4m3
        )
        xtc = np.ascontiguousarray(X[sl].T)  # [t, n] f32

        in_maps.append(
            {
                "wall": np.ascontiguousarray(wallv),
                "ohall": ohall,
                "xmov": xtc.astype(np.float16),
                "xb": np.ascontiguousarray(
                    np.concatenate([xtc, b_out[sl].T], axis=1)
                ),
            }
        )
    return in_maps


def kernel(X, W_in, b_in, W_out, b_out, col_idx):
    from concourse.bass_utils import run_bass_kernel_spmd

    nc = _build()
    in_maps = make_in_maps(X, W_in, b_in, W_out, b_out, col_idx)
    res = run_bass_kernel_spmd(nc, in_maps, list(range(M))).results
    out = np.empty((N, T), np.float32)
    for c in range(M):
        out[c * NR : (c + 1) * NR] = res[c]["out"].T
    return out
